# revision 9
# baseline (speedup 1.0000x reference)
"""BlockExtractor (GFLA/FFWM bilinear block sampler) TRN2 kernel.

Full inputs: source (8,64,128,128) f32, flow_field (8,2,128,128) f32, kernel_size=3.
Output: (8,64,384,384) f32. Data-parallel over batch: one NeuronCore per batch elem.

Per-core algorithm:
  1. Transpose source [c,h,w] -> staging [w, hp(row-padded 136), c] fp16 via PE.
  2. Build 4 row-phase DRAM copies: phase rho holds chunks (hq, w') = rows
     4hq+rho..+3 of padded col w' (cols padded by 4 via replication), each chunk
     [4 rows, 64 ch] fp16 = 512B; 4 consecutive w'-chunks = one 2KB descriptor.
  3. Per flow cell (yf,xf): one dma_gather descriptor fetches the 4x4x64 patch
     (rows t..t+3, cols a..a+3 elementwise-clamped) as [j, r, c].
  4. DVE lerps with per-partition (=per-cell) scalar weights; exact border
     handling via per-yo/per-xo weights computed from clamped corner indices.
  5. PE transposes [xf, c] -> [c, x] per output row; ACT copies PSUM->staging;
     contiguous DMA to out[c, y, x].
"""
import numpy as np
from contextlib import ExitStack

import concourse.bass as bass
import concourse.tile as tile
from concourse import bacc, mybir
from concourse import bass_utils
from concourse.tile_rust import add_dep_helper

# avoid artifact upload attempts (no bucket creds needed for plain runs)
bass_utils.upload_artifacts = lambda tmpdir: tmpdir

F32 = mybir.dt.float32
F16 = mybir.dt.float16
I32 = mybir.dt.int32
I16 = mybir.dt.int16

B, C, H, W = 8, 64, 128, 128
K = 3
NHQ = 32            # hq slots per phase copy
COPY_CHUNKS = NHQ * W         # 32*128 chunks of 256 fp16 (512B)
NCHUNKS = 4 * COPY_CHUNKS
GRP = 8             # yf per gather call (1024 idxs)
N_CORES = 8

_cache = {}


def _build():
    nc = bacc.Bacc("TRN2", target_bir_lowering=False, debug=False,
                   num_devices=N_CORES)
    src = nc.dram_tensor("source", [C, H, W], F32, kind="ExternalInput")
    flow = nc.dram_tensor("flow_field", [2, H, W], F32, kind="ExternalInput")
    out = nc.dram_tensor("out", [C, K * H, K * W], F32, kind="ExternalOutput")
    phase = nc.dram_tensor("phase", [NCHUNKS, 256], F16, kind="Internal")

    with tile.TileContext(nc) as tc, ExitStack() as ctx:
        singles = ctx.enter_context(tc.tile_pool(name="singles", bufs=1))
        psA_cm = tc.tile_pool(name="psA", bufs=2, space="PSUM")
        psA = psA_cm.__enter__()

        # ---------- identities ----------
        ones32 = singles.tile([128, 128], F32)
        nc.vector.memset(ones32[:], 1.0)
        id32 = singles.tile([128, 128], F32)
        nc.gpsimd.affine_select(id32[:], ones32[:], pattern=[[-1, 128]],
                                compare_op=mybir.AluOpType.is_equal, fill=0.0,
                                base=0, channel_multiplier=1)
        ones16 = singles.tile([128, 128], F16)
        nc.vector.memset(ones16[:], 1.0)
        id16 = singles.tile([128, 128], F16)
        nc.gpsimd.affine_select(id16[:], ones16[:], pattern=[[-1, 128]],
                                compare_op=mybir.AluOpType.is_equal, fill=0.0,
                                base=0, channel_multiplier=1)

        # ---------- load & transpose flow ----------
        flow_sb = singles.tile([128, 2, 128], F32)   # [yf, ch, xf]
        nc.sync.dma_start(flow_sb[:], flow.ap().rearrange("c y x -> y c x"))
        fT = []  # fT[ch] = [xf, yf] f32
        for ch in range(2):
            ps = psA.tile([128, 128], F32, tag="pflow")
            nc.tensor.transpose(ps[:], flow_sb[:, ch, :], id32[:])
            t = singles.tile([128, 128], F32, tag=f"fT{ch}")
            nc.vector.tensor_copy(t[:], ps[:])
            fT.append(t)
        fTy, fTx = fT

        # ---------- field computation ----------
        def floorf(pool, x, tag):
            i = pool.tile([128, 128], I32, tag=tag + "i")
            nc.vector.tensor_copy(i[:], x[:])
            f = pool.tile([128, 128], F32, tag=tag + "f")
            nc.vector.tensor_copy(f[:], i[:])
            gt = pool.tile([128, 128], F32, tag=tag + "g")
            nc.vector.tensor_tensor(out=gt[:], in0=f[:], in1=x[:],
                                    op=mybir.AluOpType.is_gt)
            nc.vector.tensor_tensor(out=f[:], in0=f[:], in1=gt[:],
                                    op=mybir.AluOpType.subtract)
            return f

        fieldp = ctx.enter_context(tc.tile_pool(name="fields", bufs=1))
        yf_i = fieldp.tile([128, 128], I32)
        nc.gpsimd.iota(yf_i[:], pattern=[[1, 128]], base=0, channel_multiplier=0)
        yfF = fieldp.tile([128, 128], F32)
        nc.vector.tensor_copy(yfF[:], yf_i[:])
        xf_i = fieldp.tile([128, 1], I32)
        nc.gpsimd.iota(xf_i[:], pattern=[[0, 1]], base=0, channel_multiplier=1)
        xfF = fieldp.tile([128, 1], F32)
        nc.vector.tensor_copy(xfF[:], xf_i[:])

        fly = floorf(fieldp, fTy, "fy")      # floor(fy)
        flx = floorf(fieldp, fTx, "fx")
        ABa = fieldp.tile([128, 128], F32)   # yf + floor(fy)
        nc.vector.tensor_add(ABa[:], yfF[:], fly[:])
        AXa = fieldp.tile([128, 128], F32)   # xf + floor(fx)
        nc.vector.tensor_scalar(out=AXa[:], in0=flx[:], scalar1=xfF[:, 0:1],
                                scalar2=None, op0=mybir.AluOpType.add)
        ysb = fieldp.tile([128, 128], F32)   # yf + fy
        nc.vector.tensor_add(ysb[:], yfF[:], fTy[:])
        xsb = fieldp.tile([128, 128], F32)   # xf + fx
        nc.vector.tensor_scalar(out=xsb[:], in0=fTx[:], scalar1=xfF[:, 0:1],
                                scalar2=None, op0=mybir.AluOpType.add)

        def clip_f(pool, x, addv, lo, hi, tag):
            r = pool.tile([128, 128], F32, tag=tag)
            nc.vector.tensor_scalar(out=r[:], in0=x[:], scalar1=float(addv),
                                    scalar2=float(lo), op0=mybir.AluOpType.add,
                                    op1=mybir.AluOpType.max)
            nc.vector.tensor_scalar(out=r[:], in0=r[:], scalar1=float(hi),
                                    scalar2=None, op0=mybir.AluOpType.min)
            return r

        t_f = clip_f(fieldp, ABa, -1, 0, 124, "t_f")   # row-window start (real)
        a_f = clip_f(fieldp, AXa, -1, 0, 124, "a_f")   # col-window start (real)

        # hq = t//4, rho = t%4  (exact in fp32)
        tq = fieldp.tile([128, 128], F32)
        nc.vector.tensor_scalar_mul(tq[:], t_f[:], 0.25)
        hqf = floorf(fieldp, tq, "hq")
        rho = fieldp.tile([128, 128], F32)
        nc.vector.tensor_scalar_mul(rho[:], hqf[:], -4.0)
        nc.vector.tensor_add(rho[:], rho[:], t_f[:])
        idxf = fieldp.tile([128, 128], F32)
        nc.vector.tensor_scalar_mul(idxf[:], rho[:], float(COPY_CHUNKS))
        tmpf = fieldp.tile([128, 128], F32)
        nc.vector.tensor_scalar_mul(tmpf[:], hqf[:], float(W))
        nc.vector.tensor_add(idxf[:], idxf[:], tmpf[:])
        nc.vector.tensor_add(idxf[:], idxf[:], a_f[:])
        idx16 = fieldp.tile([128, 128], I16)
        nc.vector.tensor_copy(idx16[:], idxf[:])

        # dense-4 window coefficients, exact incl. clamped-corner extrapolation:
        # value_o = (1-d)*src[T_o] + d*src[B_o], T_o = clip(base+o-1, 0, 127),
        # B_o = min(T_o+1, 127), d = s_o - T_o. Window rows/cols w0+r, r=0..3,
        # w0 = clip(base-1, 0, 124). coeff[o][r] = (1-d)*[w0+r==T_o] + d*[w0+r==B_o].
        def dense_coeffs(base, sfrac, w0, pfx):
            res = []
            for o in range(3):
                Tc = clip_f(fieldp, base, o - 1, 0, 127, pfx + "T")
                Bc = fieldp.tile([128, 128], F32, tag=pfx + "B")
                nc.vector.tensor_scalar(out=Bc[:], in0=Tc[:], scalar1=1.0,
                                        scalar2=127.0, op0=mybir.AluOpType.add,
                                        op1=mybir.AluOpType.min)
                sv = fieldp.tile([128, 128], F32, tag=pfx + "s")
                nc.vector.tensor_scalar(out=sv[:], in0=sfrac[:],
                                        scalar1=float(o - 1), scalar2=None,
                                        op0=mybir.AluOpType.add)
                d = fieldp.tile([128, 128], F32, tag=pfx + "d")
                nc.vector.tensor_tensor(out=d[:], in0=sv[:], in1=Tc[:],
                                        op=mybir.AluOpType.subtract)
                omd = fieldp.tile([128, 128], F32, tag=pfx + "od")
                nc.vector.tensor_scalar(out=omd[:], in0=d[:], scalar1=-1.0,
                                        scalar2=1.0, op0=mybir.AluOpType.mult,
                                        op1=mybir.AluOpType.add)
                pT = fieldp.tile([128, 128], F32, tag=pfx + "pT")
                nc.vector.tensor_tensor(out=pT[:], in0=Tc[:], in1=w0[:],
                                        op=mybir.AluOpType.subtract)
                pB = fieldp.tile([128, 128], F32, tag=pfx + "pB")
                nc.vector.tensor_tensor(out=pB[:], in0=Bc[:], in1=w0[:],
                                        op=mybir.AluOpType.subtract)
                row = []
                for r in range(4):
                    e1 = fieldp.tile([128, 128], F32, tag=pfx + "e1")
                    nc.vector.tensor_scalar(out=e1[:], in0=pT[:],
                                            scalar1=float(r), scalar2=None,
                                            op0=mybir.AluOpType.is_equal)
                    m1 = fieldp.tile([128, 128], F32, tag=pfx + "m1")
                    nc.vector.tensor_tensor(out=m1[:], in0=e1[:], in1=omd[:],
                                            op=mybir.AluOpType.mult)
                    e2 = fieldp.tile([128, 128], F32, tag=pfx + "e2")
                    nc.vector.tensor_scalar(out=e2[:], in0=pB[:],
                                            scalar1=float(r), scalar2=None,
                                            op0=mybir.AluOpType.is_equal)
                    m2 = fieldp.tile([128, 128], F32, tag=pfx + "m2")
                    nc.vector.tensor_tensor(out=m2[:], in0=e2[:], in1=d[:],
                                            op=mybir.AluOpType.mult)
                    cv = fieldp.tile([128, 128], F32, tag=f"{pfx}cv{o}{r}")
                    nc.vector.tensor_tensor(out=cv[:], in0=m1[:], in1=m2[:],
                                            op=mybir.AluOpType.add)
                    row.append(cv)
                res.append(row)
            return res

        CV = dense_coeffs(ABa, ysb, t_f, "y")   # CV[yo][r]
        CH = dense_coeffs(AXa, xsb, a_f, "x")   # CH[xo][j]

        # wrap idx16 [xf, yf] -> idxw [32, 128*8] (partition q=xf%16, slot yf*8+k)
        idx_fold = singles.tile([16, 8, 128], I16)   # [q, k, yf]
        for q in range(16):
            src_ap = bass.AP(tensor=idx16.tensor, offset=idx16.offset + q * 128,
                             ap=[[16 * 128, 8], [1, 128]])
            dst_ap = bass.AP(tensor=idx_fold.tensor,
                             offset=idx_fold.offset + q * 1024,
                             ap=[[1024, 1], [128, 8], [1, 128]])
            nc.sync.dma_start(dst_ap, src_ap)
        idxw = singles.tile([32, 128 * 8], I16)
        # interleave (k, yf) -> slot yf*8+k on DVE
        dst_ap = bass.AP(tensor=idxw.tensor, offset=idxw.offset,
                         ap=[[1024, 16], [1, 8], [8, 128]])
        nc.vector.tensor_copy(dst_ap, idx_fold[:])
        nc.sync.dma_start(idxw[16:32, :], idxw[0:16, :])

        # ---------- source load, cast, transpose to staging [w, hp, c] ----------
        stageA = tc.tile_pool(name="stageA", bufs=1)
        with stageA as sA:
            src_sb = sA.tile([C, H * W], F32)
            nc.sync.dma_start(src_sb[:], src.ap().rearrange("c h w -> c (h w)"))
            src16 = sA.tile([C, H, W], F16)
            # split cast between DVE and ACT
            nc.vector.tensor_copy(src16[:, 0:64, :],
                                  src_sb[:, 0:64 * W].rearrange("c (h w) -> c h w", w=W))
            nc.scalar.copy(src16[:, 64:128, :],
                           src_sb[:, 64 * W:].rearrange("c (h w) -> c h w", w=W))

            staging = singles.tile([128, H, C], F16)
            for h in range(H):
                ps = psA.tile([128, C], F16, tag="ptr")
                nc.tensor.transpose(ps[:], src16[:, h, :], id16[0:C, 0:C])
                if h % 2 == 0:
                    nc.scalar.copy(staging[:, h, :], ps[:])
                else:
                    nc.vector.tensor_copy(staging[:, h, :], ps[:])

        psA_cm.__exit__(None, None, None)

        # ---------- phase-copy build DMAs ----------
        build_insts = []
        for rho_i in range(4):
            cnt = 32 if rho_i == 0 else 31
            src_ap = bass.AP(tensor=staging.tensor,
                             offset=staging.offset + rho_i * C,
                             ap=[staging.ap[0]] + [[4 * C, cnt], [1, 4 * C]])
            dst_ap = bass.AP(tensor=phase,
                             offset=(rho_i * COPY_CHUNKS) * 256,
                             ap=[[256, 128], [W * 256, cnt], [1, 256]])
            bi = nc.sync.dma_start(dst_ap, src_ap)
            build_insts.append(bi)

        # ---------- main loop ----------
        mainp = ctx.enter_context(tc.tile_pool(name="main", bufs=2))
        psC = ctx.enter_context(tc.tile_pool(name="psC", bufs=4, space="PSUM"))
        outp = ctx.enter_context(tc.tile_pool(name="outs", bufs=3))

        in_view = bass.AP(tensor=phase, offset=0, ap=[[256, NCHUNKS - 3], [1, 1024]])
        for grp in range(H // GRP):
            patches = mainp.tile([128, GRP, 4, 4, C], F16, tag="patches")
            gi = nc.gpsimd.dma_gather(
                patches[:].rearrange("p g j r c -> p g (j r c)"), in_view,
                idxw[:, grp * GRP * 8:(grp + 1) * GRP * 8],
                GRP * 128, GRP * 128, 1024, elem_step=256)
            for b in build_insts:
                add_dep_helper(gi.ins, b.ins, reason="gather reads phase copies")

            for g in range(GRP):
                yf = grp * GRP + g
                V = mainp.tile([128, 3, 4, C], F16, tag="V")
                for yo in range(3):
                    t1 = mainp.tile([128, 4, C], F16, tag="vt1")
                    nc.vector.tensor_scalar(
                        out=t1[:], in0=patches[:, g, :, 0, :],
                        scalar1=CV[yo][0][:, yf:yf + 1], scalar2=None,
                        op0=mybir.AluOpType.mult)
                    for r in range(1, 4):
                        t2 = mainp.tile([128, 4, C], F16,
                                        tag="vt2" if r % 2 else "vt3")
                        nc.vector.affine_then_add(
                            out=(V[:, yo] if r == 3 else t2[:]),
                            in0=patches[:, g, :, r, :], in1=t1[:],
                            scale=CV[yo][r][:, yf:yf + 1], bias=0.0)
                        t1 = t2
                O = mainp.tile([128, 3, 3, C], F16, tag="O")
                for xo in range(3):
                    h1 = mainp.tile([128, 3, C], F16, tag="ht1")
                    nc.vector.tensor_scalar(
                        out=h1[:], in0=V[:, :, 0, :],
                        scalar1=CH[xo][0][:, yf:yf + 1], scalar2=None,
                        op0=mybir.AluOpType.mult)
                    for j in range(1, 4):
                        h2 = mainp.tile([128, 3, C], F16,
                                        tag="ht2" if j % 2 else "ht3")
                        nc.vector.affine_then_add(
                            out=(O[:, :, xo, :] if j == 3 else h2[:]),
                            in0=V[:, :, j, :], in1=h1[:],
                            scale=CH[xo][j][:, yf:yf + 1], bias=0.0)
                        h1 = h2
                ost = outp.tile([C, 3, K * W], F32, tag="ost")
                for yo in range(3):
                    ps = psC.tile([C, 3, 128], F16, tag="pout")
                    for xo in range(3):
                        nc.tensor.transpose(ps[:, xo, :], O[:, yo, xo, :], id16[:])
                    dst = bass.AP(tensor=ost.tensor,
                                  offset=ost.offset + yo * (K * W),
                                  ap=[ost.ap[0], [1, 3], [3, 128]])
                    nc.scalar.copy(dst, ps[:])
                nc.sync.dma_start(out.ap()[:, 3 * yf:3 * yf + 3, :], ost[:])

    nc.compile()
    return nc


def kernel(source, flow_field, kernel_size):
    assert int(kernel_size) == K
    source = np.ascontiguousarray(np.asarray(source, dtype=np.float32))
    flow_field = np.ascontiguousarray(np.asarray(flow_field, dtype=np.float32))
    assert source.shape == (B, C, H, W) and flow_field.shape == (B, 2, H, W)

    if "nc" not in _cache:
        _cache["nc"] = _build()
    nc = _cache["nc"]

    in_maps = [{"source": source[b], "flow_field": flow_field[b]}
               for b in range(B)]
    res = bass_utils.run_bass_kernel_spmd(
        nc, in_maps, core_ids=list(range(N_CORES)), trace=False)
    return np.stack([res.results[b]["out"] for b in range(B)], axis=0)


def run_traced(source, flow_field, kernel_size):
    """Like kernel() but with NTFF tracing; returns (output, BassKernelResults)."""
    assert int(kernel_size) == K
    source = np.ascontiguousarray(np.asarray(source, dtype=np.float32))
    flow_field = np.ascontiguousarray(np.asarray(flow_field, dtype=np.float32))
    if "nc" not in _cache:
        _cache["nc"] = _build()
    nc = _cache["nc"]
    in_maps = [{"source": source[b], "flow_field": flow_field[b]}
               for b in range(B)]
    res = bass_utils.run_bass_kernel_spmd(
        nc, in_maps, core_ids=list(range(N_CORES)), trace=True)
    out = np.stack([res.results[b]["out"] for b in range(B)], axis=0)
    return out, res


# revision 10
# speedup vs baseline: 1.2079x; 1.2079x over previous
"""BlockExtractor (GFLA/FFWM bilinear block sampler) TRN2 kernel.

Full inputs: source (8,64,128,128) f32, flow_field (8,2,128,128) f32, kernel_size=3.
Output: (8,64,384,384) f32. Data-parallel over batch: one NeuronCore per batch elem.

Per-core algorithm:
  1. Transpose source [c,h,w] -> staging [w, hp(row-padded 136), c] fp16 via PE.
  2. Build 4 row-phase DRAM copies: phase rho holds chunks (hq, w') = rows
     4hq+rho..+3 of padded col w' (cols padded by 4 via replication), each chunk
     [4 rows, 64 ch] fp16 = 512B; 4 consecutive w'-chunks = one 2KB descriptor.
  3. Per flow cell (yf,xf): one dma_gather descriptor fetches the 4x4x64 patch
     (rows t..t+3, cols a..a+3 elementwise-clamped) as [j, r, c].
  4. DVE lerps with per-partition (=per-cell) scalar weights; exact border
     handling via per-yo/per-xo weights computed from clamped corner indices.
  5. PE transposes [xf, c] -> [c, x] per output row; ACT copies PSUM->staging;
     contiguous DMA to out[c, y, x].
"""
import numpy as np
from contextlib import ExitStack

import concourse.bass as bass
import concourse.tile as tile
from concourse import bacc, mybir
from concourse import bass_utils
from concourse.tile_rust import add_dep_helper

# avoid artifact upload attempts (no bucket creds needed for plain runs)
bass_utils.upload_artifacts = lambda tmpdir: tmpdir

F32 = mybir.dt.float32
F16 = mybir.dt.float16
I32 = mybir.dt.int32
I16 = mybir.dt.int16

B, C, H, W = 8, 64, 128, 128
K = 3
NHQ = 32            # hq slots per phase copy
COPY_CHUNKS = NHQ * W         # 32*128 chunks of 256 fp16 (512B)
NCHUNKS = 4 * COPY_CHUNKS
GRP = 8             # yf per gather call (1024 idxs)
N_CORES = 8

_cache = {}


def _build():
    nc = bacc.Bacc("TRN2", target_bir_lowering=False, debug=False,
                   num_devices=N_CORES)
    src = nc.dram_tensor("source", [C, H, W], F32, kind="ExternalInput")
    flow = nc.dram_tensor("flow_field", [2, H, W], F32, kind="ExternalInput")
    out = nc.dram_tensor("out", [C, K * H, K * W], F32, kind="ExternalOutput")
    phase = nc.dram_tensor("phase", [NCHUNKS, 256], F16, kind="Internal")

    with tile.TileContext(nc) as tc, ExitStack() as ctx:
        singles = ctx.enter_context(tc.tile_pool(name="singles", bufs=1))
        psA_cm = tc.tile_pool(name="psA", bufs=2, space="PSUM")
        psA = psA_cm.__enter__()

        # ---------- identities ----------
        ones32 = singles.tile([128, 128], F32)
        nc.vector.memset(ones32[:], 1.0)
        id32 = singles.tile([128, 128], F32)
        nc.gpsimd.affine_select(id32[:], ones32[:], pattern=[[-1, 128]],
                                compare_op=mybir.AluOpType.is_equal, fill=0.0,
                                base=0, channel_multiplier=1)
        ones16 = singles.tile([128, 128], F16)
        nc.vector.memset(ones16[:], 1.0)
        id16 = singles.tile([128, 128], F16)
        nc.gpsimd.affine_select(id16[:], ones16[:], pattern=[[-1, 128]],
                                compare_op=mybir.AluOpType.is_equal, fill=0.0,
                                base=0, channel_multiplier=1)

        # ---------- load & transpose flow ----------
        flow_sb = singles.tile([128, 2, 128], F32)   # [yf, ch, xf]
        nc.sync.dma_start(flow_sb[:], flow.ap().rearrange("c y x -> y c x"))
        fT = []  # fT[ch] = [xf, yf] f32
        for ch in range(2):
            ps = psA.tile([128, 128], F32, tag="pflow")
            nc.tensor.transpose(ps[:], flow_sb[:, ch, :], id32[:])
            t = singles.tile([128, 128], F32, tag=f"fT{ch}")
            nc.vector.tensor_copy(t[:], ps[:])
            fT.append(t)
        fTy, fTx = fT

        # ---------- field computation ----------
        def floorf(pool, x, tag):
            i = pool.tile([128, 128], I32, tag=tag + "i")
            nc.vector.tensor_copy(i[:], x[:])
            f = pool.tile([128, 128], F32, tag=tag + "f")
            nc.vector.tensor_copy(f[:], i[:])
            gt = pool.tile([128, 128], F32, tag=tag + "g")
            nc.vector.tensor_tensor(out=gt[:], in0=f[:], in1=x[:],
                                    op=mybir.AluOpType.is_gt)
            nc.vector.tensor_tensor(out=f[:], in0=f[:], in1=gt[:],
                                    op=mybir.AluOpType.subtract)
            return f

        fieldp = ctx.enter_context(tc.tile_pool(name="fields", bufs=1))
        yf_i = fieldp.tile([128, 128], I32)
        nc.gpsimd.iota(yf_i[:], pattern=[[1, 128]], base=0, channel_multiplier=0)
        yfF = fieldp.tile([128, 128], F32)
        nc.vector.tensor_copy(yfF[:], yf_i[:])
        xf_i = fieldp.tile([128, 1], I32)
        nc.gpsimd.iota(xf_i[:], pattern=[[0, 1]], base=0, channel_multiplier=1)
        xfF = fieldp.tile([128, 1], F32)
        nc.vector.tensor_copy(xfF[:], xf_i[:])

        fly = floorf(fieldp, fTy, "fy")      # floor(fy)
        flx = floorf(fieldp, fTx, "fx")
        ABa = fieldp.tile([128, 128], F32)   # yf + floor(fy)
        nc.vector.tensor_add(ABa[:], yfF[:], fly[:])
        AXa = fieldp.tile([128, 128], F32)   # xf + floor(fx)
        nc.vector.tensor_scalar(out=AXa[:], in0=flx[:], scalar1=xfF[:, 0:1],
                                scalar2=None, op0=mybir.AluOpType.add)
        ysb = fieldp.tile([128, 128], F32)   # yf + fy
        nc.vector.tensor_add(ysb[:], yfF[:], fTy[:])
        xsb = fieldp.tile([128, 128], F32)   # xf + fx
        nc.vector.tensor_scalar(out=xsb[:], in0=fTx[:], scalar1=xfF[:, 0:1],
                                scalar2=None, op0=mybir.AluOpType.add)

        def clip_f(pool, x, addv, lo, hi, tag):
            r = pool.tile([128, 128], F32, tag=tag)
            nc.vector.tensor_scalar(out=r[:], in0=x[:], scalar1=float(addv),
                                    scalar2=float(lo), op0=mybir.AluOpType.add,
                                    op1=mybir.AluOpType.max)
            nc.vector.tensor_scalar(out=r[:], in0=r[:], scalar1=float(hi),
                                    scalar2=None, op0=mybir.AluOpType.min)
            return r

        t_f = clip_f(fieldp, ABa, -1, 0, 124, "t_f")   # row-window start (real)
        a_f = clip_f(fieldp, AXa, -1, 0, 124, "a_f")   # col-window start (real)

        # hq = t//4, rho = t%4  (exact in fp32)
        tq = fieldp.tile([128, 128], F32)
        nc.vector.tensor_scalar_mul(tq[:], t_f[:], 0.25)
        hqf = floorf(fieldp, tq, "hq")
        rho = fieldp.tile([128, 128], F32)
        nc.vector.tensor_scalar_mul(rho[:], hqf[:], -4.0)
        nc.vector.tensor_add(rho[:], rho[:], t_f[:])
        idxf = fieldp.tile([128, 128], F32)
        nc.vector.tensor_scalar_mul(idxf[:], rho[:], float(COPY_CHUNKS))
        tmpf = fieldp.tile([128, 128], F32)
        nc.vector.tensor_scalar_mul(tmpf[:], hqf[:], float(W))
        nc.vector.tensor_add(idxf[:], idxf[:], tmpf[:])
        nc.vector.tensor_add(idxf[:], idxf[:], a_f[:])
        idx16 = fieldp.tile([128, 128], I16)
        nc.vector.tensor_copy(idx16[:], idxf[:])

        # dense-4 window coefficients, exact incl. clamped-corner extrapolation:
        # value_o = (1-d)*src[T_o] + d*src[B_o], T_o = clip(base+o-1, 0, 127),
        # B_o = min(T_o+1, 127), d = s_o - T_o. Window rows/cols w0+r, r=0..3,
        # w0 = clip(base-1, 0, 124). coeff[o][r] = (1-d)*[w0+r==T_o] + d*[w0+r==B_o].
        def dense_coeffs(base, sfrac, w0, pfx):
            res = []
            for o in range(3):
                Tc = clip_f(fieldp, base, o - 1, 0, 127, pfx + "T")
                Bc = fieldp.tile([128, 128], F32, tag=pfx + "B")
                nc.vector.tensor_scalar(out=Bc[:], in0=Tc[:], scalar1=1.0,
                                        scalar2=127.0, op0=mybir.AluOpType.add,
                                        op1=mybir.AluOpType.min)
                sv = fieldp.tile([128, 128], F32, tag=pfx + "s")
                nc.vector.tensor_scalar(out=sv[:], in0=sfrac[:],
                                        scalar1=float(o - 1), scalar2=None,
                                        op0=mybir.AluOpType.add)
                d = fieldp.tile([128, 128], F32, tag=pfx + "d")
                nc.vector.tensor_tensor(out=d[:], in0=sv[:], in1=Tc[:],
                                        op=mybir.AluOpType.subtract)
                omd = fieldp.tile([128, 128], F32, tag=pfx + "od")
                nc.vector.tensor_scalar(out=omd[:], in0=d[:], scalar1=-1.0,
                                        scalar2=1.0, op0=mybir.AluOpType.mult,
                                        op1=mybir.AluOpType.add)
                pT = fieldp.tile([128, 128], F32, tag=pfx + "pT")
                nc.vector.tensor_tensor(out=pT[:], in0=Tc[:], in1=w0[:],
                                        op=mybir.AluOpType.subtract)
                pB = fieldp.tile([128, 128], F32, tag=pfx + "pB")
                nc.vector.tensor_tensor(out=pB[:], in0=Bc[:], in1=w0[:],
                                        op=mybir.AluOpType.subtract)
                row = []
                for r in range(4):
                    e1 = fieldp.tile([128, 128], F32, tag=pfx + "e1")
                    nc.vector.tensor_scalar(out=e1[:], in0=pT[:],
                                            scalar1=float(r), scalar2=None,
                                            op0=mybir.AluOpType.is_equal)
                    m1 = fieldp.tile([128, 128], F32, tag=pfx + "m1")
                    nc.vector.tensor_tensor(out=m1[:], in0=e1[:], in1=omd[:],
                                            op=mybir.AluOpType.mult)
                    e2 = fieldp.tile([128, 128], F32, tag=pfx + "e2")
                    nc.vector.tensor_scalar(out=e2[:], in0=pB[:],
                                            scalar1=float(r), scalar2=None,
                                            op0=mybir.AluOpType.is_equal)
                    m2 = fieldp.tile([128, 128], F32, tag=pfx + "m2")
                    nc.vector.tensor_tensor(out=m2[:], in0=e2[:], in1=d[:],
                                            op=mybir.AluOpType.mult)
                    cv = fieldp.tile([128, 128], F32, tag=f"{pfx}cv{o}{r}")
                    nc.vector.tensor_tensor(out=cv[:], in0=m1[:], in1=m2[:],
                                            op=mybir.AluOpType.add)
                    row.append(cv)
                res.append(row)
            return res

        CV = dense_coeffs(ABa, ysb, t_f, "y")   # CV[yo][r]
        CH = dense_coeffs(AXa, xsb, a_f, "x")   # CH[xo][j]

        # wrap idx16 [xf, yf] -> idxw [32, 128*8] (partition q=xf%16, slot yf*8+k)
        idx_fold = singles.tile([16, 8, 128], I16)   # [q, k, yf]
        for q in range(16):
            src_ap = bass.AP(tensor=idx16.tensor, offset=idx16.offset + q * 128,
                             ap=[[16 * 128, 8], [1, 128]])
            dst_ap = bass.AP(tensor=idx_fold.tensor,
                             offset=idx_fold.offset + q * 1024,
                             ap=[[1024, 1], [128, 8], [1, 128]])
            nc.sync.dma_start(dst_ap, src_ap)
        idxw = singles.tile([32, 128 * 8], I16)
        # interleave (k, yf) -> slot yf*8+k on DVE
        dst_ap = bass.AP(tensor=idxw.tensor, offset=idxw.offset,
                         ap=[[1024, 16], [1, 8], [8, 128]])
        nc.vector.tensor_copy(dst_ap, idx_fold[:])
        nc.sync.dma_start(idxw[16:32, :], idxw[0:16, :])

        # ---------- source load, cast, transpose to staging [w, hp, c] ----------
        stageA = tc.tile_pool(name="stageA", bufs=1)
        with stageA as sA:
            src_sb = sA.tile([C, H * W], F32)
            nc.sync.dma_start(src_sb[:], src.ap().rearrange("c h w -> c (h w)"))
            src16 = sA.tile([C, H, W], F16)
            # split cast between DVE and ACT
            nc.vector.tensor_copy(src16[:, 0:64, :],
                                  src_sb[:, 0:64 * W].rearrange("c (h w) -> c h w", w=W))
            nc.scalar.copy(src16[:, 64:128, :],
                           src_sb[:, 64 * W:].rearrange("c (h w) -> c h w", w=W))

            staging = singles.tile([128, H, C], F16)
            for h in range(H):
                ps = psA.tile([128, C], F16, tag="ptr")
                nc.tensor.transpose(ps[:], src16[:, h, :], id16[0:C, 0:C])
                if h % 2 == 0:
                    nc.scalar.copy(staging[:, h, :], ps[:])
                else:
                    nc.vector.tensor_copy(staging[:, h, :], ps[:])

        psA_cm.__exit__(None, None, None)

        # ---------- phase-copy build DMAs ----------
        build_insts = []
        for rho_i in range(4):
            cnt = 32 if rho_i == 0 else 31
            src_ap = bass.AP(tensor=staging.tensor,
                             offset=staging.offset + rho_i * C,
                             ap=[staging.ap[0]] + [[4 * C, cnt], [1, 4 * C]])
            dst_ap = bass.AP(tensor=phase,
                             offset=(rho_i * COPY_CHUNKS) * 256,
                             ap=[[256, 128], [W * 256, cnt], [1, 256]])
            bi = nc.sync.dma_start(dst_ap, src_ap)
            build_insts.append(bi)

        # ---------- main loop ----------
        mainp = ctx.enter_context(tc.tile_pool(name="main", bufs=2))
        psC = ctx.enter_context(tc.tile_pool(name="psC", bufs=4, space="PSUM"))
        outp = ctx.enter_context(tc.tile_pool(name="outs", bufs=3))

        in_view = bass.AP(tensor=phase, offset=0, ap=[[256, NCHUNKS - 3], [1, 1024]])
        for grp in range(H // GRP):
            patches = mainp.tile([128, GRP, 4, 4, C], F16, tag="patches")
            gi = nc.gpsimd.dma_gather(
                patches[:].rearrange("p g j r c -> p g (j r c)"), in_view,
                idxw[:, grp * GRP * 8:(grp + 1) * GRP * 8],
                GRP * 128, GRP * 128, 1024, elem_step=256)
            for b in build_insts:
                add_dep_helper(gi.ins, b.ins, reason="gather reads phase copies")

            for g in range(GRP):
                yf = grp * GRP + g
                # horizontal first: contiguous patch slices [:, g, j, :, :]
                HZ = mainp.tile([128, 3, 4, C], F16, tag="HZ")  # [xo, r, c]
                for xo in range(3):
                    h1 = mainp.tile([128, 4, C], F16, tag="ht1")
                    nc.scalar.mul(h1[:], patches[:, g, 0, :, :],
                                  CH[xo][0][:, yf:yf + 1])
                    for j in range(1, 4):
                        h2 = mainp.tile([128, 4, C], F16,
                                        tag="ht2" if j % 2 else "ht3")
                        nc.vector.affine_then_add(
                            out=(HZ[:, xo] if j == 3 else h2[:]),
                            in0=patches[:, g, j, :, :], in1=h1[:],
                            scale=CH[xo][j][:, yf:yf + 1], bias=0.0)
                        h1 = h2
                # vertical: contiguous r-slices of HZ-transposed view [xo, r, c]
                O = mainp.tile([128, 3, 3, C], F16, tag="O")    # [yo, xo, c]
                for yo in range(3):
                    v1 = mainp.tile([128, 3, C], F16, tag="vt1")
                    nc.scalar.mul(v1[:], HZ[:, :, 0, :],
                                  CV[yo][0][:, yf:yf + 1])
                    for r in range(1, 4):
                        v2 = mainp.tile([128, 3, C], F16,
                                        tag="vt2" if r % 2 else "vt3")
                        nc.vector.affine_then_add(
                            out=(O[:, yo] if r == 3 else v2[:]),
                            in0=HZ[:, :, r, :], in1=v1[:],
                            scale=CV[yo][r][:, yf:yf + 1], bias=0.0)
                        v1 = v2
                ost = outp.tile([C, 3, K * W], F32, tag="ost")
                for yo in range(3):
                    ps = psC.tile([C, 3, 128], F16, tag="pout")
                    for xo in range(3):
                        nc.tensor.transpose(ps[:, xo, :], O[:, yo, xo, :], id16[:])
                    dst = bass.AP(tensor=ost.tensor,
                                  offset=ost.offset + yo * (K * W),
                                  ap=[ost.ap[0], [1, 3], [3, 128]])
                    nc.scalar.copy(dst, ps[:])
                nc.sync.dma_start(out.ap()[:, 3 * yf:3 * yf + 3, :], ost[:])

    nc.compile()
    return nc


def kernel(source, flow_field, kernel_size):
    assert int(kernel_size) == K
    source = np.ascontiguousarray(np.asarray(source, dtype=np.float32))
    flow_field = np.ascontiguousarray(np.asarray(flow_field, dtype=np.float32))
    assert source.shape == (B, C, H, W) and flow_field.shape == (B, 2, H, W)

    if "nc" not in _cache:
        _cache["nc"] = _build()
    nc = _cache["nc"]

    in_maps = [{"source": source[b], "flow_field": flow_field[b]}
               for b in range(B)]
    res = bass_utils.run_bass_kernel_spmd(
        nc, in_maps, core_ids=list(range(N_CORES)), trace=False)
    return np.stack([res.results[b]["out"] for b in range(B)], axis=0)


def run_traced(source, flow_field, kernel_size):
    """Like kernel() but with NTFF tracing; returns (output, BassKernelResults)."""
    assert int(kernel_size) == K
    source = np.ascontiguousarray(np.asarray(source, dtype=np.float32))
    flow_field = np.ascontiguousarray(np.asarray(flow_field, dtype=np.float32))
    if "nc" not in _cache:
        _cache["nc"] = _build()
    nc = _cache["nc"]
    in_maps = [{"source": source[b], "flow_field": flow_field[b]}
               for b in range(B)]
    res = bass_utils.run_bass_kernel_spmd(
        nc, in_maps, core_ids=list(range(N_CORES)), trace=True)
    out = np.stack([res.results[b]["out"] for b in range(B)], axis=0)
    return out, res
